# Initial kernel scaffold
#
"""Causal self-attention (B=4, S=2048, E=1024, H=16) on 8 TRN2 NeuronCores.

Sharding: data parallel on batch (4) x tensor parallel on heads (2 groups of 8).
Core c handles batch c//2, heads (c%2)*8..(c%2)*8+8. Each core computes its
heads' QKV projections, causal attention, and a partial output projection
(w_proj rows for its heads, with b_proj/2 folded in); core pairs then
ReduceScatter-add the partials so each core finishes half the rows of its
batch. No transposes on device: scores are computed as S^T = K @ Q^T, the
softmax denominator comes from a ones-column appended to V, and O^T stacked
over heads is exactly the lhsT the projection matmul needs.

All matmuls run in float32r (fp32 data, full-rate PE mode, ~1e-4 relative).
"""

import sys

sys.path.insert(0, "/opt/trn_rl_repo")

import numpy as np

import concourse.mybir as mybir
import concourse.tile as tile
from concourse import bacc
from concourse.bass_utils import run_bass_kernel_spmd

B, S, E, H, D = 4, 2048, 1024, 16, 64
P = 128
HPC = 8  # heads per core
NPAIR = HPC // 2  # head pairs per core (2 heads share a 128-partition tile)
HE = HPC * D  # 512: embedding slice owned by one core
NKT = S // P  # 16 k-row tiles
NKS = S // 512  # 4 sequence slices of 512
NEC = E // P  # 8 contraction chunks
DV = D + 1  # V columns per head incl. ones column
f32 = mybir.dt.float32
f32r = mybir.dt.float32r
EXPF = mybir.ActivationFunctionType.Exp

_CACHED = {}


def _build_program():
    nc = bacc.Bacc(None, target_bir_lowering=False)

    xT_d = nc.dram_tensor("xT", [E, S], f32r, kind="ExternalInput")
    wq_d = nc.dram_tensor("wq", [E, HE], f32r, kind="ExternalInput")
    wk_d = nc.dram_tensor("wk", [E, HE], f32r, kind="ExternalInput")
    wv_d = nc.dram_tensor("wv", [E, HE], f32r, kind="ExternalInput")
    wp_d = nc.dram_tensor("wp", [HE, E], f32r, kind="ExternalInput")
    bq_d = nc.dram_tensor("bq", [HE], f32, kind="ExternalInput")
    bk_d = nc.dram_tensor("bk", [HE], f32, kind="ExternalInput")
    bv_d = nc.dram_tensor("bv", [1, HE], f32r, kind="ExternalInput")
    bp_d = nc.dram_tensor("bp", [1, E], f32r, kind="ExternalInput")
    y_d = nc.dram_tensor("y_out", [S // 2, E], f32, kind="ExternalOutput")

    with tile.TileContext(nc) as tc:
        with (
            tc.tile_pool(name="const", bufs=1) as cst,
            tc.tile_pool(name="kt", bufs=1) as ktp,
            tc.tile_pool(name="qt", bufs=1) as qtp,
            tc.tile_pool(name="vx", bufs=1) as vxp,
            tc.tile_pool(name="yt", bufs=1) as ytp,
            tc.tile_pool(name="dram", bufs=1, space="DRAM") as dramp,
        ):
            # ---- constants -------------------------------------------------
            ones32 = cst.tile([P, P], f32)
            nc.gpsimd.memset(ones32[:], 1.0)
            ones_row = cst.tile([1, P], f32r)
            nc.vector.tensor_copy(ones_row[:], ones32[0:1, :])
            onescol = cst.tile([P, NPAIR * 2], f32r)
            nc.vector.tensor_copy(onescol[:], ones32[:, : NPAIR * 2])

            # diag mask [128,128]: keep (1.0) where col >= row
            dmask32 = cst.tile([P, P], f32)
            nc.gpsimd.memset(dmask32[:], 1.0)
            nc.gpsimd.affine_select(
                out=dmask32[:],
                in_=dmask32[:],
                compare_op=mybir.AluOpType.is_ge,
                fill=0.0,
                base=0,
                pattern=[[1, P]],
                channel_multiplier=-1,
            )
            dmask = cst.tile([P, P], f32r)
            nc.vector.tensor_copy(dmask[:], dmask32[:])
            # mask3 [128,256]: keep where col - 128 >= row (zeros | diag)
            m3_32 = cst.tile([P, 256], f32)
            nc.gpsimd.memset(m3_32[:], 1.0)
            nc.gpsimd.affine_select(
                out=m3_32[:],
                in_=m3_32[:],
                compare_op=mybir.AluOpType.is_ge,
                fill=0.0,
                base=-128,
                pattern=[[1, 256]],
                channel_multiplier=-1,
            )
            mask3 = cst.tile([P, 256], f32r)
            nc.vector.tensor_copy(mask3[:], m3_32[:])

            bq_sb = cst.tile([P, NPAIR], f32)
            nc.sync.dma_start(bq_sb[:], bq_d.rearrange("(o p) -> p o", p=P))
            bk_sb = cst.tile([P, NPAIR], f32)
            nc.sync.dma_start(bk_sb[:], bk_d.rearrange("(o p) -> p o", p=P))
            bv_row = cst.tile([1, HE], f32r)
            nc.sync.dma_start(bv_row[:], bv_d[:])
            bp_row = cst.tile([1, E], f32r)
            nc.sync.dma_start(bp_row[:], bp_d[:])

            # persistent activations
            kT = [ktp.tile([P, S], f32r, tag=f"kt{p}") for p in range(NPAIR)]
            qT = [qtp.tile([P, S], f32r, tag=f"qt{p}") for p in range(NPAIR)]
            vx = [vxp.tile([P, HPC * DV], f32r, tag=f"vx{k}") for k in range(NKT)]
            yT = [ytp.tile([P, S], f32r, tag=f"yt{p}") for p in range(NPAIR)]

            # ---- bias broadcast rows -> [128, *] tiles via K=1 matmul ------
            with tc.tile_pool(name="ps_bias", bufs=1, space="PSUM") as psb:
                bv_bc = cst.tile([P, HE], f32)
                bp_bc = cst.tile([P, E], f32)
                bvp = psb.tile([P, HE], f32, tag="bv")
                nc.tensor.matmul(bvp[:], ones_row[:], bv_row[:], start=True, stop=True)
                nc.vector.tensor_copy(bv_bc[:], bvp[:])
                for i in range(2):
                    bpp = psb.tile([P, 512], f32, tag="bp")
                    nc.tensor.matmul(
                        bpp[:],
                        ones_row[:],
                        bp_row[:, 512 * i : 512 * (i + 1)],
                        start=True,
                        stop=True,
                    )
                    nc.vector.tensor_copy(bp_bc[:, 512 * i : 512 * (i + 1)], bpp[:])

            # ---- phase 1: QKV projections ---------------------------------
            with (
                tc.tile_pool(name="xs", bufs=2) as xsp,
                tc.tile_pool(name="wgt", bufs=1) as wgp,
                tc.tile_pool(name="ps_qkv", bufs=2, space="PSUM") as psq,
            ):
                # K^T and Q^T: out [128 (pair d), 512 seq] per (pair, slice)
                for name, w_dram, b_sb, dst in (
                    ("k", wk_d, bk_sb, kT),
                    ("q", wq_d, bq_sb, qT),
                ):
                    w_sb = wgp.tile([P, NEC, HE], f32r, tag="w")
                    nc.sync.dma_start(
                        w_sb[:], w_dram.rearrange("(o p) m -> p o m", p=P)
                    )
                    for ks in range(NKS):
                        xs = xsp.tile([P, NEC, 512], f32r, tag="xs")
                        nc.sync.dma_start(
                            xs[:],
                            xT_d[:, 512 * ks : 512 * (ks + 1)].rearrange(
                                "(o p) s -> p o s", p=P
                            ),
                        )
                        for p in range(NPAIR):
                            acc = psq.tile([P, 512], f32, tag=f"a{p}")
                            for o in range(NEC):
                                nc.tensor.matmul(
                                    acc[:],
                                    w_sb[:, o, P * p : P * (p + 1)],
                                    xs[:, o, :],
                                    start=(o == 0),
                                    stop=(o == NEC - 1),
                                )
                            # evac + per-partition bias (broadcast along free)
                            nc.vector.tensor_tensor(
                                dst[p][:, 512 * ks : 512 * (ks + 1)],
                                acc[:],
                                b_sb[:, p : p + 1].to_broadcast((P, 512)),
                                mybir.AluOpType.add,
                            )

                # V: out [128 k-rows, 512 d] per k-tile; scatter into vx + ones
                wv_sb = wgp.tile([P, NEC, HE], f32r, tag="w")
                nc.sync.dma_start(wv_sb[:], wv_d.rearrange("(o p) m -> p o m", p=P))
                for kt_i in range(NKT):
                    xv = xsp.tile([P, NEC, P], f32r, tag="xv")
                    nc.sync.dma_start(
                        xv[:],
                        xT_d[:, P * kt_i : P * (kt_i + 1)].rearrange(
                            "(o p) s -> p o s", p=P
                        ),
                    )
                    acc = psq.tile([P, HE], f32, tag="v")
                    for o in range(NEC):
                        nc.tensor.matmul(
                            acc[:],
                            xv[:, o, :],
                            wv_sb[:, o, :],
                            start=(o == 0),
                            stop=(o == NEC - 1),
                        )
                    vt = vx[kt_i].rearrange("p (h d) -> p h d", d=DV)
                    nc.vector.tensor_tensor(
                        vt[:, :, 0:D],
                        acc[:].rearrange("p (h d) -> p h d", d=D),
                        bv_bc[:].rearrange("p (h d) -> p h d", d=D),
                        mybir.AluOpType.add,
                    )
                    nc.vector.tensor_copy(vt[:, :, D], onescol[:])

            # ---- phase 2: attention ---------------------------------------
            with (
                tc.tile_pool(name="pt", bufs=3) as ptp,
                tc.tile_pool(name="rc", bufs=2) as rcp,
                tc.tile_pool(name="ps_s", bufs=3, space="PSUM") as pss,
                tc.tile_pool(name="ps_o", bufs=2, space="PSUM") as pso,
                tc.tile_pool(name="ps_bc", bufs=2, space="PSUM") as psc,
            ):
                # staircase: (start col, mask tile, mask col) per sub-diagonal t
                stair = (
                    (0, "d", 0),
                    (128, "d", 128),
                    (256, "d", 256),
                    (256, "m3", 256),
                )
                for p in range(NPAIR):
                    for h in range(2):
                        hl = 2 * p + h
                        for tau in range(NKS):
                            qs = 512 * tau
                            q_ap = qT[p][64 * h : 64 * h + 64, qs : qs + 512]
                            o_ps = pso.tile([P, 512], f32, tag="o")
                            n_chunks = 4 * tau + 4
                            for j in range(n_chunks):
                                t = j - 4 * tau  # >=0 on the staircase
                                c0 = 0 if t < 0 else stair[t][0]
                                w = 512 - c0
                                s_ps = pss.tile([P, 512], f32, tag="s")
                                nc.tensor.matmul(
                                    s_ps[:, c0:512],
                                    kT[p][64 * h : 64 * h + 64, P * j : P * (j + 1)],
                                    q_ap[:, c0:512],
                                    start=True,
                                    stop=True,
                                )
                                p_sb = ptp.tile([P, 512], f32r, tag="p")
                                nc.scalar.activation(
                                    p_sb[:, c0:512], s_ps[:, c0:512], EXPF
                                )
                                if t >= 0:
                                    _, mk, mc = stair[t]
                                    mt = dmask if mk == "d" else mask3
                                    mw = P if mk == "d" else 256
                                    nc.vector.tensor_mul(
                                        p_sb[:, mc : mc + mw],
                                        p_sb[:, mc : mc + mw],
                                        mt[:, :mw],
                                    )
                                nc.tensor.matmul(
                                    o_ps[0:DV, c0:512],
                                    vx[j][:, DV * hl : DV * (hl + 1)],
                                    p_sb[:, c0:512],
                                    start=(j == 0),
                                    stop=(j == n_chunks - 1),
                                )
                            # softmax normalize: rows 0..63 / row 64
                            rc = rcp.tile([1, 512], f32r, tag="rc")
                            with nc.allow_low_precision(reason="softmax recip"):
                                nc.vector.reciprocal(rc[:], o_ps[64:65, :])
                            bc_ps = psc.tile([P, 512], f32, tag="bc")
                            nc.tensor.matmul(
                                bc_ps[0:64, :],
                                ones_row[:, 0:64],
                                rc[:],
                                start=True,
                                stop=True,
                            )
                            nc.vector.tensor_mul(
                                yT[p][64 * h : 64 * h + 64, qs : qs + 512],
                                o_ps[0:64, :],
                                bc_ps[0:64, :],
                            )

            # ---- phase 3: output projection + ReduceScatter ----------------
            y_part = dramp.tile([S, E], f32)
            y_half = dramp.tile([S // 2, E], f32)
            with (
                tc.tile_pool(name="wp", bufs=1) as wpp,
                tc.tile_pool(name="yp", bufs=3) as ypp,
                tc.tile_pool(name="ps_pr", bufs=4, space="PSUM") as psp,
            ):
                wp_sb = wpp.tile([P, NPAIR, E], f32r)
                nc.sync.dma_start(wp_sb[:], wp_d.rearrange("(o p) m -> p o m", p=P))
                for qt_i in range(NKT):
                    yp_sb = ypp.tile([P, E], f32, tag="yp")
                    for ec in range(2):
                        acc = psp.tile([P, 512], f32, tag=f"pr{ec}")
                        for p in range(NPAIR):
                            nc.tensor.matmul(
                                acc[:],
                                yT[p][:, P * qt_i : P * (qt_i + 1)],
                                wp_sb[:, p, 512 * ec : 512 * (ec + 1)],
                                start=(p == 0),
                                stop=(p == NPAIR - 1),
                            )
                        nc.vector.tensor_add(
                            yp_sb[:, 512 * ec : 512 * (ec + 1)],
                            acc[:],
                            bp_bc[:, 512 * ec : 512 * (ec + 1)],
                        )
                    nc.sync.dma_start(y_part[P * qt_i : P * (qt_i + 1), :], yp_sb[:])

            nc.gpsimd.collective_compute(
                "ReduceScatter",
                mybir.AluOpType.add,
                replica_groups=[[0, 1], [2, 3], [4, 5], [6, 7]],
                ins=[y_part.opt()],
                outs=[y_half.opt()],
            )
            nc.gpsimd.dma_start(y_d[:], y_half[:])

    nc.finalize()
    return nc


def _get_program():
    if "nc" not in _CACHED:
        _CACHED["nc"] = _build_program()
    return _CACHED["nc"]


def kernel(x, w_qkv, b_qkv, w_proj, b_proj, trace=False):
    x = np.ascontiguousarray(np.asarray(x, dtype=np.float32))
    w_qkv = np.asarray(w_qkv, dtype=np.float32)
    b_qkv = np.asarray(b_qkv, dtype=np.float32)
    w_proj = np.asarray(w_proj, dtype=np.float32)
    b_proj = np.asarray(b_proj, dtype=np.float32)

    wq, wk, wv = w_qkv[:, :E], w_qkv[:, E : 2 * E], w_qkv[:, 2 * E :]
    bq, bk, bv = b_qkv[:E], b_qkv[E : 2 * E], b_qkv[2 * E :]
    scale = 1.0 / np.sqrt(np.float32(D))

    in_maps = []
    for c in range(8):
        b, g = divmod(c, 2)
        sl = slice(g * HE, (g + 1) * HE)
        in_maps.append(
            {
                "xT": np.ascontiguousarray(x[b].T),
                "wq": np.ascontiguousarray(wq[:, sl] * scale),
                "wk": np.ascontiguousarray(wk[:, sl]),
                "wv": np.ascontiguousarray(wv[:, sl]),
                "wp": np.ascontiguousarray(w_proj[sl, :]),
                "bq": np.ascontiguousarray(bq[sl] * scale),
                "bk": np.ascontiguousarray(bk[sl]),
                "bv": np.ascontiguousarray(bv[sl][None, :]),
                "bp": np.ascontiguousarray((b_proj * 0.5)[None, :]),
            }
        )

    nc = _get_program()
    res = run_bass_kernel_spmd(nc, in_maps, list(range(8)), trace=trace)

    out = np.empty((B, S, E), dtype=np.float32)
    for c in range(8):
        b, g = divmod(c, 2)
        rows = slice(0, S // 2) if g == 0 else slice(S // 2, S)
        out[b, rows, :] = res.results[c]["y_out"]
    if trace:
        return out, res
    return out


# revision 20
# speedup vs baseline: 1.1947x; 1.1947x over previous
"""Causal self-attention (B=4, S=2048, E=1024, H=16) on 8 TRN2 NeuronCores.

Sharding: data parallel on batch (4) x tensor parallel on heads (2 groups of 8).
Core c handles batch c//2, heads (c%2)*8..(c%2)*8+8. Each core computes its
heads' QKV projections, causal attention, and a partial output projection
(w_proj rows for its heads, with b_proj/2 folded in); core pairs then
ReduceScatter-add the partials so each core finishes half the rows of its
batch. No transposes on device: scores are computed as S^T = K @ Q^T, the
softmax denominator comes from a ones-column appended to V, and O^T stacked
over heads is exactly the lhsT the projection matmul needs.

Projections run in float32r (fp32 data, full-rate PE mode, ~1e-4 relative);
attention matmuls run in bf16 (f32r is ~3x slower at K=64/M=65).
"""

import sys

sys.path.insert(0, "/opt/trn_rl_repo")

import numpy as np

import concourse.mybir as mybir
import concourse.tile as tile
from concourse import bacc
from concourse.bass_utils import run_bass_kernel_spmd

B, S, E, H, D = 4, 2048, 1024, 16, 64
P = 128
HPC = 8  # heads per core
NPAIR = HPC // 2  # head pairs per core (2 heads share a 128-partition tile)
HE = HPC * D  # 512: embedding slice owned by one core
NKT = S // P  # 16 k-row tiles
NKS = S // 512  # 4 sequence slices of 512
NEC = E // P  # 8 contraction chunks
DV = D + 1  # V columns per head incl. ones column
f32 = mybir.dt.float32
f32r = mybir.dt.float32r
bf16 = mybir.dt.bfloat16
EXPF = mybir.ActivationFunctionType.Exp

_CACHED = {}


def _build_program():
    nc = bacc.Bacc(None, target_bir_lowering=False)

    xT_d = nc.dram_tensor("xT", [E, S], f32r, kind="ExternalInput")
    wq_d = nc.dram_tensor("wq", [E, HE], f32r, kind="ExternalInput")
    wk_d = nc.dram_tensor("wk", [E, HE], f32r, kind="ExternalInput")
    wv_d = nc.dram_tensor("wv", [E, HE], f32r, kind="ExternalInput")
    wp_d = nc.dram_tensor("wp", [HE, E], f32r, kind="ExternalInput")
    bq_d = nc.dram_tensor("bq", [HE], f32, kind="ExternalInput")
    bk_d = nc.dram_tensor("bk", [HE], f32, kind="ExternalInput")
    bv_d = nc.dram_tensor("bv", [1, HE], f32r, kind="ExternalInput")
    bp_d = nc.dram_tensor("bp", [1, E], f32r, kind="ExternalInput")
    y_d = nc.dram_tensor("y_out", [S // 2, E], f32, kind="ExternalOutput")

    with tile.TileContext(nc) as tc:
        with (
            tc.tile_pool(name="const", bufs=1) as cst,
            tc.tile_pool(name="kt", bufs=1) as ktp,
            tc.tile_pool(name="qt", bufs=1) as qtp,
            tc.tile_pool(name="vx", bufs=1) as vxp,
            tc.tile_pool(name="dram", bufs=1, space="DRAM") as dramp,
        ):
            # ---- constants -------------------------------------------------
            ones32 = cst.tile([P, P], f32)
            nc.gpsimd.memset(ones32[:], 1.0)
            ones_row = cst.tile([1, P], f32r)
            nc.vector.tensor_copy(ones_row[:], ones32[0:1, :])
            ones65 = cst.tile([65, 64], f32r)
            nc.vector.tensor_copy(ones65[:], ones32[0:65, 0:64])
            onescol = cst.tile([P, NPAIR * 2], bf16)
            nc.vector.tensor_copy(onescol[:], ones32[:, : NPAIR * 2])

            # diag mask [128,128]: keep (1.0) where col >= row
            dmask32 = cst.tile([P, P], f32)
            nc.gpsimd.memset(dmask32[:], 1.0)
            nc.gpsimd.affine_select(
                out=dmask32[:],
                in_=dmask32[:],
                compare_op=mybir.AluOpType.is_ge,
                fill=0.0,
                base=0,
                pattern=[[1, P]],
                channel_multiplier=-1,
            )
            dmask = cst.tile([P, P], bf16)
            nc.vector.tensor_copy(dmask[:], dmask32[:])
            # mask3 [128,256]: keep where col - 128 >= row (zeros | diag)
            m3_32 = cst.tile([P, 256], f32)
            nc.gpsimd.memset(m3_32[:], 1.0)
            nc.gpsimd.affine_select(
                out=m3_32[:],
                in_=m3_32[:],
                compare_op=mybir.AluOpType.is_ge,
                fill=0.0,
                base=-128,
                pattern=[[1, 256]],
                channel_multiplier=-1,
            )
            mask3 = cst.tile([P, 256], bf16)
            nc.vector.tensor_copy(mask3[:], m3_32[:])

            bq_sb = cst.tile([P, NPAIR], f32)
            nc.sync.dma_start(bq_sb[:], bq_d.rearrange("(o p) -> p o", p=P))
            bk_sb = cst.tile([P, NPAIR], f32)
            nc.sync.dma_start(bk_sb[:], bk_d.rearrange("(o p) -> p o", p=P))
            bv_row = cst.tile([1, HE], f32r)
            nc.sync.dma_start(bv_row[:], bv_d[:])
            bp_row = cst.tile([1, E], f32r)
            nc.sync.dma_start(bp_row[:], bp_d[:])

            # persistent activations
            kT = [ktp.tile([P, S], bf16, tag=f"kt{p}", name=f"kt{p}") for p in range(NPAIR)]
            qT = [qtp.tile([P, S], bf16, tag=f"qt{p}", name=f"qt{p}") for p in range(NPAIR)]
            vx = [vxp.tile([P, HPC * DV], bf16, tag=f"vx{k}", name=f"vx{k}") for k in range(NKT)]

            # ---- bias broadcast rows -> [128, *] tiles via K=1 matmul ------
            with tc.tile_pool(name="ps_bias", bufs=1, space="PSUM") as psb:
                bv_bc = cst.tile([P, HE], f32)
                bp_bc = cst.tile([P, E], f32)
                bvp = psb.tile([P, HE], f32, tag="bv")
                nc.tensor.matmul(bvp[:], ones_row[:], bv_row[:], start=True, stop=True)
                nc.vector.tensor_copy(bv_bc[:], bvp[:])
                for i in range(2):
                    bpp = psb.tile([P, 512], f32, tag="bp")
                    nc.tensor.matmul(
                        bpp[:],
                        ones_row[:],
                        bp_row[:, 512 * i : 512 * (i + 1)],
                        start=True,
                        stop=True,
                    )
                    nc.vector.tensor_copy(bp_bc[:, 512 * i : 512 * (i + 1)], bpp[:])

            # ---- phase 1: QKV projections ---------------------------------
            with (
                tc.tile_pool(name="xs", bufs=2) as xsp,
                tc.tile_pool(name="wgt", bufs=2) as wgp,
                tc.tile_pool(name="ps_qkv", bufs=3, space="PSUM") as psq,
            ):
                # K^T and Q^T: out [128 (pair d), 512 seq] per (pair, slice)
                for name, w_dram, b_sb, dst in (
                    ("k", wk_d, bk_sb, kT),
                    ("q", wq_d, bq_sb, qT),
                ):
                    w_sb = wgp.tile([P, NEC, HE], f32r, tag="w")
                    nc.sync.dma_start(
                        w_sb[:], w_dram.rearrange("(o p) m -> p o m", p=P)
                    )
                    for ks in range(NKS):
                        xs = xsp.tile([P, NEC, 512], f32r, tag="xs")
                        nc.sync.dma_start(
                            xs[:],
                            xT_d[:, 512 * ks : 512 * (ks + 1)].rearrange(
                                "(o p) s -> p o s", p=P
                            ),
                        )
                        for p in range(NPAIR):
                            acc = psq.tile([P, 512], f32, tag="a")
                            for o in range(NEC):
                                nc.tensor.matmul(
                                    acc[:],
                                    w_sb[:, o, P * p : P * (p + 1)],
                                    xs[:, o, :],
                                    start=(o == 0),
                                    stop=(o == NEC - 1),
                                )
                            # evac + per-partition bias (broadcast along free)
                            nc.vector.tensor_tensor(
                                dst[p][:, 512 * ks : 512 * (ks + 1)],
                                acc[:],
                                b_sb[:, p : p + 1].to_broadcast((P, 512)),
                                mybir.AluOpType.add,
                            )

                # V: out [128 k-rows, 512 d] per k-tile; scatter into vx + ones
                wv_sb = wgp.tile([P, NEC, HE], f32r, tag="w")
                nc.sync.dma_start(wv_sb[:], wv_d.rearrange("(o p) m -> p o m", p=P))
                for kt_i in range(NKT):
                    xv = xsp.tile([P, NEC, P], f32r, tag="xv")
                    nc.sync.dma_start(
                        xv[:],
                        xT_d[:, P * kt_i : P * (kt_i + 1)].rearrange(
                            "(o p) s -> p o s", p=P
                        ),
                    )
                    acc = psq.tile([P, HE], f32, tag="v")
                    for o in range(NEC):
                        nc.tensor.matmul(
                            acc[:],
                            xv[:, o, :],
                            wv_sb[:, o, :],
                            start=(o == 0),
                            stop=(o == NEC - 1),
                        )
                    vt = vx[kt_i].rearrange("p (h d) -> p h d", d=DV)
                    nc.vector.tensor_tensor(
                        vt[:, :, 0:D],
                        acc[:].rearrange("p (h d) -> p h d", d=D),
                        bv_bc[:].rearrange("p (h d) -> p h d", d=D),
                        mybir.AluOpType.add,
                    )
                    nc.vector.tensor_copy(vt[:, :, D], onescol[:])

            # ---- phase 2: attention ---------------------------------------
            with tc.tile_pool(name="yt", bufs=1) as ytp:
                yT = [ytp.tile([P, S], f32r, tag=f"yt{p}", name=f"yt{p}") for p in range(NPAIR)]
                with (
                    tc.tile_pool(name="pt", bufs=3) as ptp,
                    tc.tile_pool(name="sm", bufs=3) as smp,
                    tc.tile_pool(name="os", bufs=4) as osp,
                    tc.tile_pool(name="ps_s", bufs=2, space="PSUM") as pss,
                    tc.tile_pool(name="ps_o", bufs=3, space="PSUM") as pso,
                    tc.tile_pool(name="ps_bc", bufs=1, space="PSUM") as psc,
                ):
                    # staircase: (start col, mask tile, mask col) per sub-diag t
                    stair = (
                        (0, "d", 0),
                        (128, "d", 128),
                        (256, "d", 256),
                        (256, "m3", 256),
                    )
                    # normalize in batches of 2: denominators gathered at
                    # partitions 0 and 64 (quadrant-aligned for the broadcast
                    # matmul rhs), one reciprocal per batch
                    pending = []

                    def flush_normalize():
                        dn = smp.tile([65, 512], f32r, tag="dn", name="dn")
                        for i, (o_sb, dst) in enumerate(pending):
                            nc.vector.tensor_copy(
                                dn[64 * i : 64 * i + 1, :], o_sb[64:65, :]
                            )
                        with nc.allow_low_precision(reason="softmax recip"):
                            if len(pending) == 2:
                                # rows 1..63 hold garbage, recip'd harmlessly
                                nc.vector.reciprocal(dn[0:65, :], dn[0:65, :])
                            else:
                                nc.vector.reciprocal(dn[0:1, :], dn[0:1, :])
                        for i, (o_sb, dst) in enumerate(pending):
                            bc_ps = psc.tile([P, 512], f32, tag="bc")
                            nc.tensor.matmul(
                                bc_ps[0:64, :],
                                ones65[64 * i : 64 * i + 1, :],
                                dn[64 * i : 64 * i + 1, :],
                                start=True,
                                stop=True,
                            )
                            bc_sb = smp.tile([64, 512], f32, tag="bcs")
                            nc.vector.tensor_copy(bc_sb[:], bc_ps[0:64, :])
                            nc.vector.tensor_mul(dst, o_sb[0:64, :], bc_sb[:])
                        pending.clear()

                    for p in range(NPAIR):
                        for h in range(2):
                            hl = 2 * p + h
                            for tau in range(NKS):
                                qs = 512 * tau
                                q_ap = qT[p][64 * h : 64 * h + 64, qs : qs + 512]
                                o_ps = pso.tile([P, 512], f32, tag="o")
                                n_chunks = 4 * tau + 4
                                # chunks processed in pairs sharing a 2-bank
                                # psum tile so one Exp covers both
                                for a in range(0, n_chunks, 2):
                                    cols = []
                                    for idx in range(2):
                                        j = a + idx
                                        t = j - 4 * tau
                                        cols.append(0 if t < 0 else stair[t][0])
                                    s2 = pss.tile([P, 1024], f32, tag="s")
                                    p2 = ptp.tile([P, 1024], bf16, tag="p")
                                    for idx in range(2):
                                        j = a + idx
                                        off = 512 * idx
                                        c0 = cols[idx]
                                        nc.tensor.matmul(
                                            s2[:, off + c0 : off + 512],
                                            kT[p][
                                                64 * h : 64 * h + 64,
                                                P * j : P * (j + 1),
                                            ],
                                            q_ap[:, c0:512],
                                            start=True,
                                            stop=True,
                                        )
                                    # one Exp over both chunks (the gap cols
                                    # hold stale psum, exp'd harmlessly)
                                    nc.scalar.activation(
                                        p2[:, cols[0] : 1024],
                                        s2[:, cols[0] : 1024],
                                        EXPF,
                                    )
                                    for idx in range(2):
                                        j = a + idx
                                        t = j - 4 * tau
                                        off = 512 * idx
                                        c0 = cols[idx]
                                        if t >= 0:
                                            _, mk, mc = stair[t]
                                            mt = dmask if mk == "d" else mask3
                                            mw = P if mk == "d" else 256
                                            nc.vector.tensor_mul(
                                                p2[:, off + mc : off + mc + mw],
                                                p2[:, off + mc : off + mc + mw],
                                                mt[:, :mw],
                                            )
                                        nc.tensor.matmul(
                                            o_ps[0:DV, c0:512],
                                            vx[j][:, DV * hl : DV * (hl + 1)],
                                            p2[:, off + c0 : off + 512],
                                            start=(j == 0),
                                            stop=(j == n_chunks - 1),
                                        )
                                # evacuate O^T (and its denom row) to SBUF,
                                # freeing the psum bank before normalization
                                o_sb = osp.tile([DV, 512], f32, tag="os")
                                nc.vector.tensor_copy(o_sb[:], o_ps[0:DV, :])
                                pending.append(
                                    (o_sb, yT[p][64 * h : 64 * h + 64, qs : qs + 512])
                                )
                                if len(pending) == 2:
                                    flush_normalize()
                    if pending:
                        flush_normalize()
                # ---- phase 3: output projection + chunked ReduceScatter ----
                y_parts = [dramp.tile([512, E], f32, name=f"ypart{j}") for j in range(4)]
                y_halves = [dramp.tile([256, E], f32, name=f"yhalf{j}") for j in range(4)]
                with (
                    tc.tile_pool(name="wp", bufs=1) as wpp,
                    tc.tile_pool(name="yp", bufs=3) as ypp,
                    tc.tile_pool(name="ps_pr", bufs=2, space="PSUM") as psp,
                ):
                    wp_sb = wpp.tile([P, NPAIR, E], f32r)
                    nc.sync.dma_start(
                        wp_sb[:], wp_d.rearrange("(o p) m -> p o m", p=P)
                    )
                    for qt_i in range(NKT):
                        yp_sb = ypp.tile([P, E], f32, tag="yp")
                        for ec in range(2):
                            acc = psp.tile([P, 512], f32, tag=f"pr{ec}")
                            for p in range(NPAIR):
                                nc.tensor.matmul(
                                    acc[:],
                                    yT[p][:, P * qt_i : P * (qt_i + 1)],
                                    wp_sb[:, p, 512 * ec : 512 * (ec + 1)],
                                    start=(p == 0),
                                    stop=(p == NPAIR - 1),
                                )
                            nc.vector.tensor_add(
                                yp_sb[:, 512 * ec : 512 * (ec + 1)],
                                acc[:],
                                bp_bc[:, 512 * ec : 512 * (ec + 1)],
                            )
                        j = qt_i // 4
                        nc.sync.dma_start(
                            y_parts[j][P * (qt_i % 4) : P * (qt_i % 4 + 1), :],
                            yp_sb[:],
                        )
                        if qt_i % 4 == 3:
                            nc.gpsimd.collective_compute(
                                "ReduceScatter",
                                mybir.AluOpType.add,
                                replica_groups=[[0, 1], [2, 3], [4, 5], [6, 7]],
                                ins=[y_parts[j].opt()],
                                outs=[y_halves[j].opt()],
                            )
                            nc.gpsimd.dma_start(
                                y_d[256 * j : 256 * (j + 1), :], y_halves[j][:]
                            )

    nc.finalize()
    return nc


def _get_program():
    if "nc" not in _CACHED:
        _CACHED["nc"] = _build_program()
    return _CACHED["nc"]


def kernel(x, w_qkv, b_qkv, w_proj, b_proj, trace=False):
    x = np.ascontiguousarray(np.asarray(x, dtype=np.float32))
    w_qkv = np.asarray(w_qkv, dtype=np.float32)
    b_qkv = np.asarray(b_qkv, dtype=np.float32)
    w_proj = np.asarray(w_proj, dtype=np.float32)
    b_proj = np.asarray(b_proj, dtype=np.float32)

    wq, wk, wv = w_qkv[:, :E], w_qkv[:, E : 2 * E], w_qkv[:, 2 * E :]
    bq, bk, bv = b_qkv[:E], b_qkv[E : 2 * E], b_qkv[2 * E :]
    scale = 1.0 / np.sqrt(np.float32(D))

    in_maps = []
    for c in range(8):
        b, g = divmod(c, 2)
        sl = slice(g * HE, (g + 1) * HE)
        in_maps.append(
            {
                "xT": np.ascontiguousarray(x[b].T),
                "wq": np.ascontiguousarray(wq[:, sl] * scale),
                "wk": np.ascontiguousarray(wk[:, sl]),
                "wv": np.ascontiguousarray(wv[:, sl]),
                "wp": np.ascontiguousarray(w_proj[sl, :]),
                "bq": np.ascontiguousarray(bq[sl] * scale),
                "bk": np.ascontiguousarray(bk[sl]),
                "bv": np.ascontiguousarray(bv[sl][None, :]),
                "bp": np.ascontiguousarray((b_proj * 0.5)[None, :]),
            }
        )

    nc = _get_program()
    res = run_bass_kernel_spmd(nc, in_maps, list(range(8)), trace=trace)

    out = np.empty((B, S, E), dtype=np.float32)
    for c in range(8):
        b, g = divmod(c, 2)
        yo = res.results[c]["y_out"]
        # chunk j of this core's output = global rows 512*j + 256*g ..+256
        for j in range(4):
            out[b, 512 * j + 256 * g : 512 * j + 256 * (g + 1), :] = yo[
                256 * j : 256 * (j + 1)
            ]
    if trace:
        return out, res
    return out


# revision 21
# speedup vs baseline: 1.4125x; 1.1824x over previous
"""Causal self-attention (B=4, S=2048, E=1024, H=16) on 8 TRN2 NeuronCores.

Sharding: data parallel on batch (4) x tensor parallel on heads (2 groups of 8).
Core c handles batch c//2, heads (c%2)*8..(c%2)*8+8. Each core computes its
heads' QKV projections, causal attention, and a partial output projection
(w_proj rows for its heads, with b_proj/2 folded in); core pairs then
ReduceScatter-add the partials so each core finishes half the rows of its
batch. No transposes on device: scores are computed as S^T = K @ Q^T, the
softmax denominator comes from a ones-column appended to V, and O^T stacked
over heads is exactly the lhsT the projection matmul needs.

Projections run in float32r (fp32 data, full-rate PE mode, ~1e-4 relative);
attention matmuls run in bf16 (f32r is ~3x slower at K=64/M=65).
"""

import sys

sys.path.insert(0, "/opt/trn_rl_repo")

import numpy as np

import concourse.mybir as mybir
import concourse.tile as tile
from concourse import bacc
from concourse.bass_utils import run_bass_kernel_spmd

B, S, E, H, D = 4, 2048, 1024, 16, 64
P = 128
HPC = 8  # heads per core
NPAIR = HPC // 2  # head pairs per core (2 heads share a 128-partition tile)
HE = HPC * D  # 512: embedding slice owned by one core
NKT = S // P  # 16 k-row tiles
NKS = S // 512  # 4 sequence slices of 512
NEC = E // P  # 8 contraction chunks
DV = D + 1  # V columns per head incl. ones column
f32 = mybir.dt.float32
f32r = mybir.dt.float32r
bf16 = mybir.dt.bfloat16
EXPF = mybir.ActivationFunctionType.Exp

_CACHED = {}


def _build_program():
    nc = bacc.Bacc(None, target_bir_lowering=False)

    xT_d = nc.dram_tensor("xT", [E, S], f32r, kind="ExternalInput")
    wq_d = nc.dram_tensor("wq", [E, HE], f32r, kind="ExternalInput")
    wk_d = nc.dram_tensor("wk", [E, HE], f32r, kind="ExternalInput")
    wv_d = nc.dram_tensor("wv", [E, HE], f32r, kind="ExternalInput")
    wp_d = nc.dram_tensor("wp", [HE, E], f32r, kind="ExternalInput")
    bq_d = nc.dram_tensor("bq", [HE], f32, kind="ExternalInput")
    bk_d = nc.dram_tensor("bk", [HE], f32, kind="ExternalInput")
    bv_d = nc.dram_tensor("bv", [1, HE], f32r, kind="ExternalInput")
    bp_d = nc.dram_tensor("bp", [1, E], f32r, kind="ExternalInput")
    y_d = nc.dram_tensor("y_out", [S // 2, E], f32, kind="ExternalOutput")

    with tile.TileContext(nc) as tc:
        with (
            tc.tile_pool(name="const", bufs=1) as cst,
            tc.tile_pool(name="kt", bufs=1) as ktp,
            tc.tile_pool(name="qt", bufs=1) as qtp,
            tc.tile_pool(name="vx", bufs=1) as vxp,
            tc.tile_pool(name="dram", bufs=1, space="DRAM") as dramp,
        ):
            # ---- constants -------------------------------------------------
            ones32 = cst.tile([P, P], f32)
            nc.gpsimd.memset(ones32[:], 1.0)
            ones_row = cst.tile([1, P], f32r)
            nc.vector.tensor_copy(ones_row[:], ones32[0:1, :])
            ones65 = cst.tile([65, 64], f32r)
            nc.vector.tensor_copy(ones65[:], ones32[0:65, 0:64])
            onescol = cst.tile([P, NPAIR * 2], bf16)
            nc.vector.tensor_copy(onescol[:], ones32[:, : NPAIR * 2])

            # diag mask [128,128]: keep (1.0) where col >= row
            dmask32 = cst.tile([P, P], f32)
            nc.gpsimd.memset(dmask32[:], 1.0)
            nc.gpsimd.affine_select(
                out=dmask32[:],
                in_=dmask32[:],
                compare_op=mybir.AluOpType.is_ge,
                fill=0.0,
                base=0,
                pattern=[[1, P]],
                channel_multiplier=-1,
            )
            dmask = cst.tile([P, P], bf16)
            nc.vector.tensor_copy(dmask[:], dmask32[:])
            # mask3 [128,256]: keep where col - 128 >= row (zeros | diag)
            m3_32 = cst.tile([P, 256], f32)
            nc.gpsimd.memset(m3_32[:], 1.0)
            nc.gpsimd.affine_select(
                out=m3_32[:],
                in_=m3_32[:],
                compare_op=mybir.AluOpType.is_ge,
                fill=0.0,
                base=-128,
                pattern=[[1, 256]],
                channel_multiplier=-1,
            )
            mask3 = cst.tile([P, 256], bf16)
            nc.vector.tensor_copy(mask3[:], m3_32[:])

            bq_sb = cst.tile([P, NPAIR], f32)
            nc.sync.dma_start(bq_sb[:], bq_d.rearrange("(o p) -> p o", p=P))
            bk_sb = cst.tile([P, NPAIR], f32)
            nc.sync.dma_start(bk_sb[:], bk_d.rearrange("(o p) -> p o", p=P))
            bv_row = cst.tile([1, HE], f32r)
            nc.sync.dma_start(bv_row[:], bv_d[:])
            bp_row = cst.tile([1, E], f32r)
            nc.sync.dma_start(bp_row[:], bp_d[:])

            # persistent activations
            kT = [ktp.tile([P, S], bf16, tag=f"kt{p}", name=f"kt{p}") for p in range(NPAIR)]
            qT = [qtp.tile([P, S], bf16, tag=f"qt{p}", name=f"qt{p}") for p in range(NPAIR)]
            vx = [vxp.tile([P, HPC * DV], bf16, tag=f"vx{k}", name=f"vx{k}") for k in range(NKT)]

            # ---- bias broadcast rows -> [128, *] tiles via K=1 matmul ------
            with tc.tile_pool(name="ps_bias", bufs=1, space="PSUM") as psb:
                bv_bc = cst.tile([P, HE], f32)
                bp_bc = cst.tile([P, E], f32)
                bvp = psb.tile([P, HE], f32, tag="bv")
                nc.tensor.matmul(bvp[:], ones_row[:], bv_row[:], start=True, stop=True)
                nc.vector.tensor_copy(bv_bc[:], bvp[:])
                for i in range(2):
                    bpp = psb.tile([P, 512], f32, tag="bp")
                    nc.tensor.matmul(
                        bpp[:],
                        ones_row[:],
                        bp_row[:, 512 * i : 512 * (i + 1)],
                        start=True,
                        stop=True,
                    )
                    nc.vector.tensor_copy(bp_bc[:, 512 * i : 512 * (i + 1)], bpp[:])

            # ---- phase 1: QKV projections ---------------------------------
            with (
                tc.tile_pool(name="xs", bufs=2) as xsp,
                tc.tile_pool(name="wgt", bufs=2) as wgp,
                tc.tile_pool(name="ps_qkv", bufs=3, space="PSUM") as psq,
            ):
                # K^T and Q^T: out [128 (pair d), 512 seq] per (pair, slice)
                for name, w_dram, b_sb, dst in (
                    ("k", wk_d, bk_sb, kT),
                    ("q", wq_d, bq_sb, qT),
                ):
                    w_sb = wgp.tile([P, NEC, HE], f32r, tag="w")
                    nc.sync.dma_start(
                        w_sb[:], w_dram.rearrange("(o p) m -> p o m", p=P)
                    )
                    for ks in range(NKS):
                        xs = xsp.tile([P, NEC, 512], f32r, tag="xs")
                        nc.sync.dma_start(
                            xs[:],
                            xT_d[:, 512 * ks : 512 * (ks + 1)].rearrange(
                                "(o p) s -> p o s", p=P
                            ),
                        )
                        for p in range(NPAIR):
                            acc = psq.tile([P, 512], f32, tag="a")
                            for o in range(NEC):
                                nc.tensor.matmul(
                                    acc[:],
                                    w_sb[:, o, P * p : P * (p + 1)],
                                    xs[:, o, :],
                                    start=(o == 0),
                                    stop=(o == NEC - 1),
                                )
                            # evac + per-partition bias (broadcast along free)
                            nc.vector.tensor_tensor(
                                dst[p][:, 512 * ks : 512 * (ks + 1)],
                                acc[:],
                                b_sb[:, p : p + 1].to_broadcast((P, 512)),
                                mybir.AluOpType.add,
                            )

                # V: out [128 k-rows, 512 d] per k-tile; scatter into vx + ones
                wv_sb = wgp.tile([P, NEC, HE], f32r, tag="w")
                nc.sync.dma_start(wv_sb[:], wv_d.rearrange("(o p) m -> p o m", p=P))
                for kt_i in range(NKT):
                    xv = xsp.tile([P, NEC, P], f32r, tag="xv")
                    nc.sync.dma_start(
                        xv[:],
                        xT_d[:, P * kt_i : P * (kt_i + 1)].rearrange(
                            "(o p) s -> p o s", p=P
                        ),
                    )
                    acc = psq.tile([P, HE], f32, tag="v")
                    for o in range(NEC):
                        nc.tensor.matmul(
                            acc[:],
                            xv[:, o, :],
                            wv_sb[:, o, :],
                            start=(o == 0),
                            stop=(o == NEC - 1),
                        )
                    vt = vx[kt_i].rearrange("p (h d) -> p h d", d=DV)
                    nc.vector.tensor_tensor(
                        vt[:, :, 0:D],
                        acc[:].rearrange("p (h d) -> p h d", d=D),
                        bv_bc[:].rearrange("p (h d) -> p h d", d=D),
                        mybir.AluOpType.add,
                    )
                    nc.vector.tensor_copy(vt[:, :, D], onescol[:])

            # ---- phases 2+3: attention, projection, ReduceScatter ----------
            # tau-outer: each 512-row q-slice finishes attention, projects,
            # and ReduceScatters while the next slice is still computing.
            y_parts = [dramp.tile([512, E], f32, name=f"ypart{j}") for j in range(4)]
            y_halves = [dramp.tile([256, E], f32, name=f"yhalf{j}") for j in range(4)]
            with tc.tile_pool(name="yt", bufs=1) as ytp:
                yT = [ytp.tile([P, S], f32r, tag=f"yt{p}", name=f"yt{p}") for p in range(NPAIR)]
                with (
                    tc.tile_pool(name="pt", bufs=4) as ptp,
                    tc.tile_pool(name="sm", bufs=3) as smp,
                    tc.tile_pool(name="os", bufs=4) as osp,
                    tc.tile_pool(name="wp", bufs=1) as wpp,
                    tc.tile_pool(name="yp", bufs=3) as ypp,
                    tc.tile_pool(name="ps_s", bufs=2, space="PSUM") as pss,
                    tc.tile_pool(name="ps_o", bufs=2, space="PSUM") as pso,
                    tc.tile_pool(name="ps_bc", bufs=1, space="PSUM") as psc,
                    tc.tile_pool(name="ps_pr", bufs=1, space="PSUM") as psp,
                ):
                    wp_sb = wpp.tile([P, NPAIR, E], f32r)
                    nc.sync.dma_start(
                        wp_sb[:], wp_d.rearrange("(o p) m -> p o m", p=P)
                    )
                    # staircase: (start col, mask tile, mask col) per sub-diag t
                    stair = (
                        (0, "d", 0),
                        (128, "d", 128),
                        (256, "d", 256),
                        (256, "m3", 256),
                    )
                    # normalize in batches of 2: denominators gathered at
                    # partitions 0 and 64 (quadrant-aligned for the broadcast
                    # matmul rhs), one reciprocal per batch
                    pending = []

                    def flush_normalize():
                        dn = smp.tile([65, 512], f32r, tag="dn", name="dn")
                        for i, (o_sb, dst) in enumerate(pending):
                            nc.vector.tensor_copy(
                                dn[64 * i : 64 * i + 1, :], o_sb[64:65, :]
                            )
                        with nc.allow_low_precision(reason="softmax recip"):
                            if len(pending) == 2:
                                # rows 1..63 hold garbage, recip'd harmlessly
                                nc.vector.reciprocal(dn[0:65, :], dn[0:65, :])
                            else:
                                nc.vector.reciprocal(dn[0:1, :], dn[0:1, :])
                        for i, (o_sb, dst) in enumerate(pending):
                            bc_ps = psc.tile([P, 512], f32, tag="bc")
                            nc.tensor.matmul(
                                bc_ps[0:64, :],
                                ones65[64 * i : 64 * i + 1, :],
                                dn[64 * i : 64 * i + 1, :],
                                start=True,
                                stop=True,
                            )
                            bc_sb = smp.tile([64, 512], f32, tag="bcs")
                            nc.vector.tensor_copy(bc_sb[:], bc_ps[0:64, :])
                            nc.vector.tensor_mul(dst, o_sb[0:64, :], bc_sb[:])
                        pending.clear()

                    for tau in range(NKS):
                        qs = 512 * tau
                        n_chunks = 4 * tau + 4
                        for p in range(NPAIR):
                            for h in range(2):
                                hl = 2 * p + h
                                q_ap = qT[p][64 * h : 64 * h + 64, qs : qs + 512]
                                o_ps = pso.tile([P, 512], f32, tag="o")
                                # chunks processed in pairs sharing a 2-bank
                                # psum tile so one Exp covers both
                                for a in range(0, n_chunks, 2):
                                    cols = []
                                    for idx in range(2):
                                        j = a + idx
                                        t = j - 4 * tau
                                        cols.append(0 if t < 0 else stair[t][0])
                                    s2 = pss.tile([P, 1024], f32, tag="s")
                                    p2 = ptp.tile([P, 1024], bf16, tag="p")
                                    for idx in range(2):
                                        j = a + idx
                                        off = 512 * idx
                                        c0 = cols[idx]
                                        nc.tensor.matmul(
                                            s2[:, off + c0 : off + 512],
                                            kT[p][
                                                64 * h : 64 * h + 64,
                                                P * j : P * (j + 1),
                                            ],
                                            q_ap[:, c0:512],
                                            start=True,
                                            stop=True,
                                        )
                                    # one Exp over both chunks (the gap cols
                                    # hold stale psum, exp'd harmlessly)
                                    nc.scalar.activation(
                                        p2[:, cols[0] : 1024],
                                        s2[:, cols[0] : 1024],
                                        EXPF,
                                    )
                                    for idx in range(2):
                                        j = a + idx
                                        t = j - 4 * tau
                                        off = 512 * idx
                                        c0 = cols[idx]
                                        if t >= 0:
                                            _, mk, mc = stair[t]
                                            mt = dmask if mk == "d" else mask3
                                            mw = P if mk == "d" else 256
                                            nc.vector.tensor_mul(
                                                p2[:, off + mc : off + mc + mw],
                                                p2[:, off + mc : off + mc + mw],
                                                mt[:, :mw],
                                            )
                                        nc.tensor.matmul(
                                            o_ps[0:DV, c0:512],
                                            vx[j][:, DV * hl : DV * (hl + 1)],
                                            p2[:, off + c0 : off + 512],
                                            start=(j == 0),
                                            stop=(j == n_chunks - 1),
                                        )
                                # evacuate O^T (and its denom row) to SBUF,
                                # freeing the psum bank before normalization
                                o_sb = osp.tile([DV, 512], f32, tag="os")
                                nc.vector.tensor_copy(o_sb[:], o_ps[0:DV, :])
                                pending.append(
                                    (
                                        o_sb,
                                        yT[p][64 * h : 64 * h + 64, qs : qs + 512],
                                    )
                                )
                                if len(pending) == 2:
                                    flush_normalize()
                        if pending:
                            flush_normalize()

                        # ---- projection + ReduceScatter for this q-slice ----
                        for ql in range(4):
                            qt_i = 4 * tau + ql
                            yp_sb = ypp.tile([P, E], f32, tag="yp")
                            for ec in range(2):
                                acc = psp.tile([P, 512], f32, tag="pr")
                                for p in range(NPAIR):
                                    nc.tensor.matmul(
                                        acc[:],
                                        yT[p][:, P * qt_i : P * (qt_i + 1)],
                                        wp_sb[:, p, 512 * ec : 512 * (ec + 1)],
                                        start=(p == 0),
                                        stop=(p == NPAIR - 1),
                                    )
                                nc.vector.tensor_add(
                                    yp_sb[:, 512 * ec : 512 * (ec + 1)],
                                    acc[:],
                                    bp_bc[:, 512 * ec : 512 * (ec + 1)],
                                )
                            nc.sync.dma_start(
                                y_parts[tau][P * ql : P * (ql + 1), :], yp_sb[:]
                            )
                        nc.gpsimd.collective_compute(
                            "ReduceScatter",
                            mybir.AluOpType.add,
                            replica_groups=[[0, 1], [2, 3], [4, 5], [6, 7]],
                            ins=[y_parts[tau].opt()],
                            outs=[y_halves[tau].opt()],
                        )
                        nc.gpsimd.dma_start(
                            y_d[256 * tau : 256 * (tau + 1), :], y_halves[tau][:]
                        )

    nc.finalize()
    return nc


def _get_program():
    if "nc" not in _CACHED:
        _CACHED["nc"] = _build_program()
    return _CACHED["nc"]


def kernel(x, w_qkv, b_qkv, w_proj, b_proj, trace=False):
    x = np.ascontiguousarray(np.asarray(x, dtype=np.float32))
    w_qkv = np.asarray(w_qkv, dtype=np.float32)
    b_qkv = np.asarray(b_qkv, dtype=np.float32)
    w_proj = np.asarray(w_proj, dtype=np.float32)
    b_proj = np.asarray(b_proj, dtype=np.float32)

    wq, wk, wv = w_qkv[:, :E], w_qkv[:, E : 2 * E], w_qkv[:, 2 * E :]
    bq, bk, bv = b_qkv[:E], b_qkv[E : 2 * E], b_qkv[2 * E :]
    scale = 1.0 / np.sqrt(np.float32(D))

    in_maps = []
    for c in range(8):
        b, g = divmod(c, 2)
        sl = slice(g * HE, (g + 1) * HE)
        in_maps.append(
            {
                "xT": np.ascontiguousarray(x[b].T),
                "wq": np.ascontiguousarray(wq[:, sl] * scale),
                "wk": np.ascontiguousarray(wk[:, sl]),
                "wv": np.ascontiguousarray(wv[:, sl]),
                "wp": np.ascontiguousarray(w_proj[sl, :]),
                "bq": np.ascontiguousarray(bq[sl] * scale),
                "bk": np.ascontiguousarray(bk[sl]),
                "bv": np.ascontiguousarray(bv[sl][None, :]),
                "bp": np.ascontiguousarray((b_proj * 0.5)[None, :]),
            }
        )

    nc = _get_program()
    res = run_bass_kernel_spmd(nc, in_maps, list(range(8)), trace=trace)

    out = np.empty((B, S, E), dtype=np.float32)
    for c in range(8):
        b, g = divmod(c, 2)
        yo = res.results[c]["y_out"]
        # chunk j of this core's output = global rows 512*j + 256*g ..+256
        for j in range(4):
            out[b, 512 * j + 256 * g : 512 * j + 256 * (g + 1), :] = yo[
                256 * j : 256 * (j + 1)
            ]
    if trace:
        return out, res
    return out
